# revision 6
# baseline (speedup 1.0000x reference)
"""AssociativeEmbeddingLoss on 8 TRN2 NeuronCores (Bass/Tile kernel).

Entry point: kernel(**inputs) -> np.ndarray of shape (3,) =
(pull_loss, push_loss, scale_loss), matching the reference.

Sharding: data-parallel on batch dim N=16 -> 2 images per core
(tags/joints/box_scales sharded on dim 0, scale_dist replicated); the
final three scalar means are reduced on the host from the per-image
partials each core returns (the "all-reduced means" gather step).

Per-core kernel design:
  - The loss touches tags only at 60 persons x 17 joints gathered rows
    of 16 floats, so instead of streaming the 8.9MB tags shard the
    kernel indirect-DMA-gathers just those ~65KB.
  - Indirect DMA costs ~1.4us per CALL (one offset per partition), so
    two joint columns are packed per call on 128 partitions (persons
    duplicated at partition offset 64): 9 calls instead of 17.
  - Partition halves merge via one PE matmul against a 0/1 selector.
  - Everything not dependent on the gathered tags (visibility counts,
    reciprocals, per-image n_val chain, box-scale target norms) runs
    hidden under the gather stream; the gathered-data statistics are
    chunked so DVE overlaps the remaining gather calls.
  - The pairwise push term is one 18x60 @ 18x60 PE matmul + Exp
    activation, with invalid persons masked by a +BIG additive term
    folded into the feature vectors (exp(-BIG) == 0).
"""

import numpy as np

import concourse.bacc as bacc
import concourse.mybir as mybir
import concourse.tile as tile
from concourse.bass import IndirectOffsetOnAxis
from concourse.bass_utils import run_bass_kernel_spmd

F32 = mybir.dt.float32
I32 = mybir.dt.int32
AF = mybir.ActivationFunctionType

S = 16  # scale-embedding dim
K = 17  # joints
M = 30  # persons per image
N = 16  # batch
L = 69632  # flattened tag locations per image (17*256*256/16)
N_CORES = 8
N_IMG = N // N_CORES  # images per core
BIG = 1e4


def _consts_np(n_img):
    J = n_img * M
    ident = np.eye(J, dtype=np.float32)
    img = np.arange(J) // M
    maskp = (img[:, None] == img[None, :]).astype(np.float32) - np.eye(J, dtype=np.float32)
    maskp = np.maximum(maskp, 0.0)
    maskp = (BIG / 2) * (1.0 - maskp)  # additive: exp(-2*x) kills masked pairs
    ind2 = np.zeros((J, n_img), np.float32)
    ind2[np.arange(J), img] = 1.0
    blob = np.concatenate([ident, maskp, ind2], axis=1)  # [J, 2J+n_img]
    # doubled-partition image offsets: persons at partitions 0..J-1 and TOP..TOP+J-1
    offs2 = np.zeros((128, 1), np.int32)
    offs2[0:J, 0] = img * L
    offs2[64 : 64 + J, 0] = img * L
    # half-merge selector: out[j] = in[j] + in[64+j] via PE
    sel = np.zeros((128, J), np.float32)
    sel[np.arange(J), np.arange(J)] = 1.0
    sel[64 + np.arange(J), np.arange(J)] = 1.0
    return blob, offs2, sel


def build_nc(n_img=2):
    J = n_img * M  # persons per core (60)
    P2 = 128  # doubled partition space; top half starts at TOP (engine APs
    TOP = 64  # may only start at partitions 0/32/64/96)
    KB = 9  # joint columns per partition half (ceil(17/2))

    nc = bacc.Bacc("TRN2", target_bir_lowering=False, debug=False)

    tags = nc.dram_tensor("tags", [n_img * L, S], F32, kind="ExternalInput")
    joints = nc.dram_tensor("joints", [128, 2 * KB], I32, kind="ExternalInput")
    bs = nc.dram_tensor("bs", [J, 1 + S], F32, kind="ExternalInput")
    out = nc.dram_tensor("out", [n_img, 3], F32, kind="ExternalOutput")

    blob_np, offs2_np, sel_np = _consts_np(n_img)
    blob_d = nc.inline_tensor(blob_np, "blob_c")
    sel_d = nc.inline_tensor(sel_np, "sel_c")

    with tile.TileContext(nc) as tc:
        with (
            tc.tile_pool(name="sb", bufs=1) as sb,
            tc.tile_pool(name="ps", bufs=1, space="PSUM") as ps,
        ):
            # ---- ACT table preloads (Abs/Sqrt/Exp) off the critical path ----
            warm = sb.tile([1, 1], F32, tag="warm")
            nc.vector.memset(warm[:], 1.0)
            w2 = sb.tile([1, 3], F32, tag="w2")
            nc.scalar.activation(out=w2[:, 0:1], in_=warm[:], func=AF.Abs)
            nc.scalar.activation(out=w2[:, 1:2], in_=warm[:], func=AF.Sqrt)
            nc.scalar.activation(out=w2[:, 2:3], in_=warm[:], func=AF.Exp)

            # ---- loads ----
            # joints arrive pre-packed by the host sharding step as [128,18]
            # int32: partitions 0-59 = (loc,vis) pairs of joints 0-8,
            # partitions 64-123 = joints 9-16, loc already rebased into the
            # core's [2L,16] shard view; pad partitions/cols are zero.
            j2 = sb.tile([P2, 2 * KB], I32, tag="j2")
            nc.sync.dma_start(j2[:], joints.ap())

            sel_sb = sb.tile([P2, J], F32, tag="sel_sb")
            nc.scalar.dma_start(sel_sb[:], sel_d.ap())
            blob_sb = sb.tile([J, 2 * J + n_img], F32, tag="blob_sb")
            nc.scalar.dma_start(blob_sb[:], blob_d.ap())
            bs_sb = sb.tile([J, 1 + S], F32, tag="bs_sb")
            nc.scalar.dma_start(bs_sb[:], bs.ap())

            ident_sb = blob_sb[:, 0:J]
            maskp_sb = blob_sb[:, J : 2 * J]
            ind2_sb = blob_sb[:, 2 * J : 2 * J + n_img]
            box_sb = bs_sb[:, 0:1]
            sd_sb = bs_sb[:, 1 : 1 + S]

            # ---- visibility ----
            j2v = j2[:].rearrange("p (k c) -> p k c", c=2)
            visf16 = sb.tile([P2, KB * S], F32, tag="visf16")
            nc.vector.tensor_copy(
                out=visf16[:].rearrange("p (k s) -> p k s", s=S),
                in_=j2v[:, :, 1:2].to_broadcast([P2, KB, S]),
            )

            # ---- the gather ----
            G = sb.tile([P2, KB * S], F32, tag="G")
            nc.vector.memset(G[:, (KB - 1) * S : KB * S], 0.0)
            for t in [KB - 1] + list(range(KB - 1)):
                pc = P2 if t < KB - 1 else J
                nc.gpsimd.indirect_dma_start(
                    out=G[0:pc, t * S : (t + 1) * S],
                    out_offset=None,
                    in_=tags.ap(),
                    in_offset=IndirectOffsetOnAxis(
                        ap=j2[0:pc, 2 * t : 2 * t + 1], axis=0
                    ),
                )

            # ---- early chains hidden under the gather ----
            # visibility counts -> safe_cnt/recip/valid and the per-image
            # n_val finalize chain, all independent of the gathered data.
            c16b = sb.tile([P2, 1], F32, tag="c16b")
            nc.vector.reduce_sum(out=c16b[:], in_=visf16[:], axis=mybir.AxisListType.X)
            cm_ps = ps.tile([J, 1], F32, tag="cm_ps")
            nc.tensor.matmul(
                out=cm_ps[:], lhsT=sel_sb[:], rhs=c16b[:], start=True, stop=True
            )
            c16e = sb.tile([J, 1], F32, tag="c16e")
            nc.vector.tensor_copy(out=c16e[:], in_=cm_ps[:])
            safe_cnt = sb.tile([J, 1], F32, tag="safe_cnt")
            nc.vector.tensor_scalar(
                out=safe_cnt[:], in0=c16e[:], scalar1=1.0 / S, scalar2=1.0,
                op0=mybir.AluOpType.mult, op1=mybir.AluOpType.max,
            )
            recip = sb.tile([J, 1], F32, tag="recip")
            nc.vector.reciprocal(out=recip[:], in_=safe_cnt[:])
            valid = sb.tile([J, 1], F32, tag="valid")
            nc.vector.tensor_single_scalar(
                out=valid[:], in_=c16e[:], scalar=0.5 * S, op=mybir.AluOpType.is_ge
            )
            nr = sb.tile([J, 1], F32, tag="nr")
            nc.vector.tensor_scalar_mul(out=nr[:], in0=recip[:], scalar1=-1.0)
            recip2 = sb.tile([J, 1], F32, tag="recip2")
            nc.vector.tensor_mul(out=recip2[:], in0=recip[:], in1=recip[:])
            rrv0 = sb.tile([J, 1], F32, tag="rrv0")
            nc.vector.tensor_scalar_mul(out=rrv0[:], in0=recip[:], scalar1=1.0 / S)
            rrv = sb.tile([J, 1], F32, tag="rrv")
            nc.vector.tensor_mul(out=rrv[:], in0=rrv0[:], in1=valid[:])
            hv = sb.tile([J, 1], F32, tag="hv")
            nc.vector.tensor_scalar(
                out=hv[:], in0=valid[:], scalar1=-BIG / 2, scalar2=BIG / 2,
                op0=mybir.AluOpType.mult, op1=mybir.AluOpType.add,
            )

            nv_ps = ps.tile([n_img, 1], F32, tag="nv_ps")
            nc.tensor.matmul(
                out=nv_ps[:], lhsT=ind2_sb, rhs=valid[:], start=True, stop=True
            )
            nvs = sb.tile([n_img, 1], F32, tag="nvs")
            nc.vector.tensor_copy(out=nvs[:], in_=nv_ps[:])
            safe_n = sb.tile([n_img, 1], F32, tag="safe_n")
            nc.vector.tensor_scalar_max(out=safe_n[:], in0=nvs[:], scalar1=1.0)
            rn = sb.tile([n_img, 1], F32, tag="rn")
            nc.vector.reciprocal(out=rn[:], in_=safe_n[:])
            nm1 = sb.tile([n_img, 1], F32, tag="nm1")
            nc.vector.tensor_scalar_add(out=nm1[:], in0=nvs[:], scalar1=-1.0)
            ppm = sb.tile([n_img, 1], F32, tag="ppm")
            nc.vector.tensor_scalar(
                out=ppm[:], in0=nvs[:], scalar1=nm1[:], scalar2=1.0,
                op0=mybir.AluOpType.mult, op1=mybir.AluOpType.max,
            )
            rp = sb.tile([n_img, 1], F32, tag="rp")
            nc.vector.reciprocal(out=rp[:], in_=ppm[:])
            ge2 = sb.tile([n_img, 1], F32, tag="ge2")
            nc.vector.tensor_single_scalar(
                out=ge2[:], in_=nvs[:], scalar=1.5, op=mybir.AluOpType.is_ge
            )

            # scale-loss pieces that depend only on box/scale_dist
            d0 = sb.tile([J, S], F32, tag="d0")
            nc.vector.tensor_scalar(
                out=d0[:], in0=sd_sb, scalar1=box_sb, scalar2=None,
                op0=mybir.AluOpType.subtract,
            )
            gap = sb.tile([J, S], F32, tag="gap")
            nc.scalar.activation(out=gap[:], in_=d0[:], func=AF.Abs)
            gap_e = sb.tile([J, S], F32, tag="gap_e")
            nc.vector.tensor_scalar_add(out=gap_e[:], in0=gap[:], scalar1=1e-10)
            r = sb.tile([J, S], F32, tag="r")
            nc.vector.reciprocal(out=r[:], in_=gap_e[:])
            r2 = sb.tile([J, S], F32, tag="r2")
            B2 = sb.tile([J, 1], F32, tag="B2")
            nc.vector.tensor_mul(out=r2[:], in0=r[:], in1=r[:])
            nc.vector.reduce_sum(out=B2[:], in_=r2[:], axis=mybir.AxisListType.X)
            mB = sb.tile([J, 1], F32, tag="mB")
            nc.vector.tensor_scalar_max(out=mB[:], in0=B2[:], scalar1=1e-24)
            sB = sb.tile([J, 1], F32, tag="sB")
            nc.scalar.sqrt(out=sB[:], in_=mB[:])
            rB = sb.tile([J, 1], F32, tag="rB")
            nc.vector.reciprocal(out=rB[:], in_=sB[:])

            # push feature tiles whose constant columns can fill early
            Wa = sb.tile([J, 18], F32, tag="Wa")
            nc.vector.memset(Wa[:, S + 1 : S + 2], 1.0)
            Wb = sb.tile([J, 18], F32, tag="Wb")
            nc.vector.memset(Wb[:, S : S + 1], 1.0)

            # ---- per-person stats, chunked to overlap the gather ----
            # gather order is [t8, t0..t7]; process block 8 first, then
            # blocks 0-3, 4-6, and finally just block 7 so only one block's
            # worth of DVE work trails the last gather call.
            gvb = sb.tile([P2, KB * S], F32, tag="gvb")
            gq = sb.tile([P2, KB * S], F32, tag="gq")
            UA = sb.tile([P2, S], F32, tag="UA")
            UB = sb.tile([P2, S], F32, tag="UB")
            sg = sb.tile([P2, 4], F32, tag="sg")
            tA = sb.tile([P2, 2 * S], F32, tag="tA")
            tB1 = sb.tile([P2, S], F32, tag="tB1")
            for ci, (lo, hi) in [(3, (8, 9)), (0, (0, 4)), (1, (4, 7)), (2, (7, 8))]:
                sl = slice(lo * S, hi * S)
                nc.vector.tensor_mul(out=gvb[:, sl], in0=G[:, sl], in1=visf16[:, sl])
                nc.vector.tensor_mul(out=gq[:, sl], in0=gvb[:, sl], in1=gvb[:, sl])
                nc.vector.reduce_sum(
                    out=sg[:, ci : ci + 1], in_=gq[:, sl], axis=mybir.AxisListType.X
                )
                if ci == 0:
                    nc.vector.tensor_add(
                        out=tA[:], in0=gvb[:, 0 : 2 * S], in1=gvb[:, 2 * S : 4 * S]
                    )
                    nc.vector.tensor_add(
                        out=UA[:], in0=tA[:, 0:S], in1=tA[:, S : 2 * S]
                    )
                elif ci == 1:
                    nc.vector.tensor_add(
                        out=tB1[:], in0=gvb[:, 4 * S : 5 * S], in1=gvb[:, 5 * S : 6 * S]
                    )
                    nc.vector.tensor_add(
                        out=UB[:], in0=tB1[:], in1=gvb[:, 6 * S : 7 * S]
                    )

            # Pack U | sum(g^2) on 128 partitions, merge halves via PE selector.
            Hpack = sb.tile([P2, S + 1], F32, tag="Hpack")
            UAB = sb.tile([P2, S], F32, tag="UAB")
            nc.vector.tensor_add(out=UAB[:], in0=UA[:], in1=UB[:])
            U78 = sb.tile([P2, S], F32, tag="U78")
            nc.vector.tensor_add(
                out=U78[:], in0=gvb[:, 7 * S : 8 * S], in1=gvb[:, 8 * S : 9 * S]
            )
            nc.vector.tensor_add(out=Hpack[:, 0:S], in0=UAB[:], in1=U78[:])
            nc.vector.reduce_sum(
                out=Hpack[:, S : S + 1], in_=sg[:], axis=mybir.AxisListType.X
            )
            Hm = ps.tile([J, S + 1], F32, tag="Hm")
            nc.tensor.matmul(
                out=Hm[:], lhsT=sel_sb[:], rhs=Hpack[:], start=True, stop=True
            )
            HmS = sb.tile([J, S + 1], F32, tag="HmS")
            nc.vector.tensor_copy(out=HmS[:], in_=Hm[:])
            U = Hm[:, 0:S]
            Sg2 = Hm[:, S : S + 1]

            Usq = sb.tile([J, S], F32, tag="Usq")
            Q = sb.tile([J, 1], F32, tag="Q")
            nc.vector.tensor_mul(out=Usq[:], in0=U, in1=HmS[:, 0:S])
            nc.vector.reduce_sum(out=Q[:], in_=Usq[:], axis=mybir.AxisListType.X)

            # stat_in columns: 0=push rowsum, 1=pull_v, 2=ds_v
            stat_in = sb.tile([J, 3], F32, tag="stat_in")

            # ---- pull: (Sg2 - Q*recip) * recip * valid / S ----
            na = sb.tile([J, 1], F32, tag="na")
            nc.vector.tensor_scalar(
                out=na[:], in0=Q[:], scalar1=nr[:], scalar2=None,
                op0=mybir.AluOpType.mult,
            )
            b = sb.tile([J, 1], F32, tag="b")
            nc.vector.tensor_add(out=b[:], in0=na[:], in1=Sg2)
            nc.vector.tensor_scalar(
                out=stat_in[:, 1:2], in0=b[:], scalar1=rrv[:], scalar2=None,
                op0=mybir.AluOpType.mult,
            )

            # ---- scale: valid * (1 - A*rB*rC) ----
            absU = sb.tile([J, S], F32, tag="absU")
            nc.scalar.activation(out=absU[:], in_=U, func=AF.Abs)
            rA = sb.tile([J, S], F32, tag="rA")
            A = sb.tile([J, 1], F32, tag="A")
            nc.vector.tensor_mul(out=rA[:], in0=r[:], in1=absU[:])
            nc.vector.reduce_sum(out=A[:], in_=rA[:], axis=mybir.AxisListType.X)
            mQ = sb.tile([J, 1], F32, tag="mQ")
            nc.vector.tensor_scalar_max(out=mQ[:], in0=Q[:], scalar1=1e-24)
            sC = sb.tile([J, 1], F32, tag="sC")
            nc.scalar.sqrt(out=sC[:], in_=mQ[:])
            rC = sb.tile([J, 1], F32, tag="rC")
            nc.vector.reciprocal(out=rC[:], in_=sC[:])
            d12 = sb.tile([J, 1], F32, tag="d12")
            nc.vector.tensor_scalar(
                out=d12[:], in0=A[:], scalar1=rB[:], scalar2=rC[:],
                op0=mybir.AluOpType.mult, op1=mybir.AluOpType.mult,
            )
            tds = sb.tile([J, 1], F32, tag="tds")
            nc.vector.tensor_scalar(
                out=tds[:], in0=d12[:], scalar1=valid[:], scalar2=None,
                op0=mybir.AluOpType.mult,
            )
            nc.vector.tensor_sub(out=stat_in[:, 2:3], in0=valid[:], in1=tds[:])

            # ---- push: exp(-||mean_i - mean_j||^2) over valid same-image pairs ----
            nc.vector.tensor_scalar(
                out=Wb[:, 0:S], in0=U, scalar1=recip[:], scalar2=None,
                op0=mybir.AluOpType.mult,
            )
            nc.vector.tensor_scalar(
                out=Wa[:, 0:S], in0=U, scalar1=nr[:], scalar2=None,
                op0=mybir.AluOpType.mult,
            )
            Qm = sb.tile([J, 1], F32, tag="Qm")
            nc.vector.tensor_scalar(
                out=Qm[:], in0=Q[:], scalar1=recip2[:], scalar2=None,
                op0=mybir.AluOpType.mult,
            )
            h = sb.tile([J, 1], F32, tag="h")
            nc.vector.tensor_scalar(
                out=h[:], in0=Qm[:], scalar1=0.5, scalar2=hv[:],
                op0=mybir.AluOpType.mult, op1=mybir.AluOpType.add,
            )
            nc.vector.tensor_copy(out=Wa[:, S : S + 1], in_=h[:])
            nc.vector.tensor_copy(out=Wb[:, S + 1 : S + 2], in_=h[:])

            Xp = ps.tile([18, J], F32, tag="Xp")
            nc.tensor.transpose(out=Xp[:], in_=Wa[:], identity=ident_sb)
            Yp = ps.tile([18, J], F32, tag="Yp")
            nc.tensor.transpose(out=Yp[:], in_=Wb[:], identity=ident_sb)
            X = sb.tile([18, J], F32, tag="X")
            nc.vector.tensor_copy(out=X[:], in_=Xp[:])
            Y = sb.tile([18, J], F32, tag="Y")
            nc.vector.tensor_copy(out=Y[:], in_=Yp[:])

            Dhat = ps.tile([J, J], F32, tag="Dhat")
            nc.tensor.matmul(out=Dhat[:], lhsT=X[:], rhs=Y[:], start=True, stop=True)

            Dmask = sb.tile([J, J], F32, tag="Dmask")
            nc.vector.tensor_add(out=Dmask[:], in0=maskp_sb, in1=Dhat[:])
            epx = sb.tile([J, J], F32, tag="epx")
            nc.scalar.activation(
                out=epx[:], in_=Dmask[:], func=AF.Exp, scale=-2.0,
                accum_out=stat_in[:, 0:1],
            )

            # ---- per-image reduction + finalize ----
            stats_ps = ps.tile([n_img, 3], F32, tag="stats_ps")
            nc.tensor.matmul(
                out=stats_ps[:], lhsT=ind2_sb, rhs=stat_in[:], start=True, stop=True
            )
            fs = sb.tile([n_img, 3], F32, tag="fs")
            nc.vector.tensor_copy(out=fs[:], in_=stats_ps[:])

            outbuf = sb.tile([n_img, 3], F32, tag="outbuf")
            nc.vector.tensor_scalar_mul(
                out=outbuf[:, 0:3:2], in0=fs[:, 1:3], scalar1=rn[:]
            )
            t6 = sb.tile([n_img, 1], F32, tag="t6")
            nc.vector.tensor_scalar(
                out=t6[:], in0=fs[:, 0:1], scalar1=0.5, scalar2=rp[:],
                op0=mybir.AluOpType.mult, op1=mybir.AluOpType.mult,
            )
            nc.vector.tensor_mul(out=outbuf[:, 1:2], in0=t6[:], in1=ge2[:])

            nc.sync.dma_start(out.ap(), outbuf[:])

    nc.compile()
    return nc


def make_in_map(tags, joints, box_scales, scale_dist, n_img=2):
    """Per-core input map from the core's shard (numpy views of full inputs).

    joints are re-laid-out into the kernel's doubled-partition format as part
    of sharding: [128, 18] int32, partitions 0-59 = (loc,vis) pairs of joints
    0-8, partitions 64-123 = joints 9-16, with loc rebased into the shard's
    flattened [n_img*L, 16] coordinate system (+ img*L).
    """
    J = n_img * M
    sd = np.asarray(scale_dist, dtype=np.float32).reshape(1, S)
    bs = np.concatenate(
        [np.asarray(box_scales, np.float32).reshape(J, 1), np.tile(sd, (J, 1))], axis=1
    )
    jr = np.asarray(joints).reshape(J, K, 2).astype(np.int32)
    offs = (np.arange(J) // M * L).astype(np.int32)
    jr = jr.copy()
    jr[:, :, 0] += offs[:, None]
    j2 = np.zeros((128, 18), np.int32)
    j2[0:J, :] = jr[:, 0:9, :].reshape(J, 18)
    j2[64 : 64 + J, 0:16] = jr[:, 9:17, :].reshape(J, 16)
    return {
        "tags": np.ascontiguousarray(tags.reshape(n_img * L, S), dtype=np.float32),
        "joints": j2,
        "bs": np.ascontiguousarray(bs),
    }

_NC_CACHE = {}


def _get_nc():
    if "nc" not in _NC_CACHE:
        _NC_CACHE["nc"] = build_nc()
    return _NC_CACHE["nc"]


def kernel(tags, joints, box_scales, scale_dist, _trace=False):
    """Full-input entry point; shards across 8 NeuronCores and gathers."""
    tags = np.asarray(tags)
    joints = np.asarray(joints)
    box_scales = np.asarray(box_scales)
    scale_dist = np.asarray(scale_dist)

    nc = _get_nc()
    in_maps = [
        make_in_map(
            tags[N_IMG * c : N_IMG * (c + 1)],
            joints[N_IMG * c : N_IMG * (c + 1)],
            box_scales[N_IMG * c : N_IMG * (c + 1)],
            scale_dist,
        )
        for c in range(N_CORES)
    ]
    res = run_bass_kernel_spmd(
        nc, in_maps, core_ids=list(range(N_CORES)), trace=_trace
    )
    parts = np.concatenate(
        [res.results[c]["out"] for c in range(N_CORES)], axis=0
    )  # [N, 3]
    final = parts.mean(axis=0).astype(np.float32)
    if _trace:
        return final, res
    return final
